# revision 1
# baseline (speedup 1.0000x reference)
"""KGAT recommender (3-layer GNN message passing) on 8 Trainium2 NeuronCores.

Sharding: edges are sharded by destination-node range — core k owns nodes
[k*12500, (k+1)*12500) and aggregates all messages into them, so no
all-reduce is needed; each layer ends with an AllGather of the updated
(bf16) node-embedding table (plus the per-edge attention scalar s=x@Wa_top
appended to each row so edge gathers fetch it for free).

Per 128-edge chunk the attention-weighted segment-sum is computed as a
one-hot matmul: W[e, j] = (j == dst_local[e]) * att[e] built in a single
DVE tensor_scalar op, then PSUM accumulates aggT[d, n] += G[e, d]^T @ W.
"""

import os
import numpy as np
import ml_dtypes

import concourse.bacc as bacc
import concourse.bass as bass
import concourse.mybir as mybir
import concourse.tile as tile
from concourse.bass_utils import run_bass_kernel_spmd
from concourse.masks import make_identity

BF16 = ml_dtypes.bfloat16

NCORES = 8
N = 100000
U = 50000
D = 128
L = 3
P = 128
NPC = N // NCORES          # 12500 nodes per core
WPC = (NPC + P - 1) // P   # 98 windows per core
NSLAB = WPC * P            # 12544 padded rows per core
TAB = NCORES * NSLAB       # 100352 rows in the gather table
SBW = 5                    # windows per superblock

LAST_EXEC_NS = None


def _host_prep(edge_index, user_emb, item_emb, Wa, ba, Wg, bg):
    x0 = np.concatenate([np.asarray(user_emb), np.asarray(item_emb)], 0).astype(np.float32)
    Wa = np.asarray(Wa, np.float32)
    ba = np.asarray(ba, np.float32)
    Wg = np.asarray(Wg, np.float32)
    bg = np.asarray(bg, np.float32)

    src = np.asarray(edge_index[0]).astype(np.int64)
    dst = np.asarray(edge_index[1]).astype(np.int64)
    E = src.shape[0]

    core = dst // NPC
    local = dst % NPC
    w = local // P
    dloc = local % P
    cell = core * WPC + w
    order = np.argsort(cell, kind="stable")
    cell_s = cell[order]
    counts = np.bincount(cell, minlength=NCORES * WPC)
    C = int(np.ceil(counts.max() / P))      # chunks per window (uniform)
    NCHUNK = WPC * C
    starts = np.zeros(NCORES * WPC, np.int64)
    starts[1:] = np.cumsum(counts)[:-1]
    rank = np.arange(E, dtype=np.int64) - starts[cell_s]
    k_arr = cell_s // WPC
    chunk = (cell_s % WPC) * C + rank // P
    p = rank % P

    srcs = src[order]
    idx1 = np.zeros((NCORES, P, NCHUNK), np.int32)
    idx2 = np.zeros((NCORES, P, NCHUNK), np.int32)
    dla = np.full((NCORES, P, NCHUNK), 300.0, np.float32)
    tabrow = (srcs // NPC) * NSLAB + (srcs % NPC)
    idx1[k_arr, p, chunk] = tabrow.astype(np.int32)
    idx2[k_arr, p, chunk] = local[order].astype(np.int32)
    dla[k_arr, p, chunk] = dloc[order].astype(np.float32)

    # layer-0 per-node attention scalars
    s0 = x0 @ Wa[0, :D, 0] + ba[0, 0]
    t0 = x0 @ Wa[0, D:, 0]

    xslab = np.zeros((NCORES, NSLAB, 256), BF16)
    for k in range(NCORES):
        xslab[k, :NPC, :D] = x0[k * NPC:(k + 1) * NPC].astype(BF16)
        xslab[k, :NPC, D] = s0[k * NPC:(k + 1) * NPC].astype(BF16)

    xt0 = np.zeros((NCORES, P, NSLAB), BF16)
    t0a = np.zeros((NCORES, NSLAB, 1), np.float32)
    for k in range(NCORES):
        xp = np.zeros((NSLAB, D), np.float32)
        xp[:NPC] = x0[k * NPC:(k + 1) * NPC]
        xt0[k] = np.ascontiguousarray(xp.T).astype(BF16)
        t0a[k, :NPC, 0] = t0[k * NPC:(k + 1) * NPC]

    wg_b = np.zeros((L, 2, D, D), BF16)
    for l in range(L):
        wg_b[l, 0] = Wg[l, :D].astype(BF16)
        wg_b[l, 1] = Wg[l, D:].astype(BF16)
    wast = np.zeros((L - 1, D, 2), BF16)
    for l in range(1, L):
        wast[l - 1, :, 0] = Wa[l, :D, 0].astype(BF16)
        wast[l - 1, :, 1] = Wa[l, D:, 0].astype(BF16)
    bg_c = bg.reshape(L, D, 1).astype(np.float32)

    return dict(C=C, NCHUNK=NCHUNK, idx1=idx1, idx2=idx2, dla=dla, xslab=xslab,
                xt0=xt0, t0a=t0a, wg_b=wg_b, wast=wast, bg_c=bg_c, ba=ba)


def _build_nc(C, NCHUNK, ba):
    L_RUN = int(os.environ.get("KGAT_LAYERS", str(L)))
    dt = mybir.dt
    nc = bacc.Bacc("TRN2", target_bir_lowering=False, debug=False,
                   enable_asserts=False, num_devices=NCORES)

    i_xslab = nc.dram_tensor("xslab", [NSLAB, 256], dt.bfloat16, kind="ExternalInput")
    i_xt0 = nc.dram_tensor("xt0", [P, NSLAB], dt.bfloat16, kind="ExternalInput")
    i_t0 = nc.dram_tensor("t0", [NSLAB, 1], dt.float32, kind="ExternalInput")
    i_idx1 = nc.dram_tensor("idx1", [P, NCHUNK], dt.int32, kind="ExternalInput")
    i_idx2 = nc.dram_tensor("idx2", [P, NCHUNK], dt.int32, kind="ExternalInput")
    i_dla = nc.dram_tensor("dla", [P, NCHUNK], dt.float32, kind="ExternalInput")
    i_wg = nc.dram_tensor("wg", [L, 2, D, D], dt.bfloat16, kind="ExternalInput")
    i_wast = nc.dram_tensor("wast", [L - 1, D, 2], dt.bfloat16, kind="ExternalInput")
    i_bg = nc.dram_tensor("bg", [L, D, 1], dt.float32, kind="ExternalInput")
    o_out = nc.dram_tensor("out", [NSLAB, D], dt.float32, kind="ExternalOutput")

    agin = [nc.dram_tensor(f"agin{l}", [NSLAB, 256], dt.bfloat16, kind="Internal")
            for l in range(L)]
    xfull = [nc.dram_tensor(f"xfull{l}", [TAB, 256], dt.bfloat16, kind="Internal",
                            addr_space="Shared")
             for l in range(L)]
    tbl = [nc.dram_tensor(f"tbl{l}", [NSLAB, 1], dt.float32, kind="Internal")
           for l in range(L - 1)]

    with tile.TileContext(nc) as tc:
        with (
            tc.tile_pool(name="sb", bufs=1) as sb,
            tc.tile_pool(name="sbg", bufs=2) as sbg,
            tc.tile_pool(name="sbw", bufs=3) as sbw,
            tc.tile_pool(name="ps", bufs=2, space="PSUM") as ps,
            tc.tile_pool(name="ps1", bufs=1, space="PSUM") as ps1,
        ):
            # ---- constants / persistent state ----
            iota_i = sb.tile([P, P], dt.int32)
            nc.gpsimd.iota(iota_i[:], pattern=[[1, P]], base=0, channel_multiplier=0)
            iota_f = sb.tile([P, P], dt.float32)
            nc.vector.tensor_copy(out=iota_f[:], in_=iota_i[:])
            ident_b = sb.tile([P, P], dt.bfloat16)
            make_identity(nc, ident_b[:])
            ident_f = sb.tile([P, P], dt.float32)
            make_identity(nc, ident_f[:])

            idx1_sb = sb.tile([P, NCHUNK], dt.int32)
            nc.sync.dma_start(out=idx1_sb[:], in_=i_idx1.ap())
            idx2_sb = sb.tile([P, NCHUNK], dt.int32)
            nc.sync.dma_start(out=idx2_sb[:], in_=i_idx2.ap())
            dla_sb = sb.tile([P, NCHUNK], dt.float32)
            nc.sync.dma_start(out=dla_sb[:], in_=i_dla.ap())

            wg_sb = sb.tile([P, L * 2 * D], dt.bfloat16)
            for l in range(L):
                for h in range(2):
                    nc.sync.dma_start(out=wg_sb[:, (l * 2 + h) * D:(l * 2 + h + 1) * D],
                                      in_=i_wg.ap()[l, h])
            wast_sb = sb.tile([P, (L - 1) * 2], dt.bfloat16)
            for l in range(L - 1):
                nc.sync.dma_start(out=wast_sb[:, l * 2:l * 2 + 2], in_=i_wast.ap()[l])
            bg_sb = sb.tile([P, L], dt.float32)
            for l in range(L):
                nc.sync.dma_start(out=bg_sb[:, l:l + 1], in_=i_bg.ap()[l])

            xt_own = sb.tile([P, NSLAB], dt.bfloat16)
            nc.sync.dma_start(out=xt_own[:], in_=i_xt0.ap())

            # replicate the layer-0 table: own slab -> AllGather
            nc.sync.dma_start(out=agin[0].ap(), in_=i_xslab.ap())
            nc.gpsimd.collective_compute(
                "AllGather", mybir.AluOpType.bypass,
                replica_groups=[list(range(NCORES))],
                ins=[agin[0].ap()], outs=[xfull[0].ap()])

            xsrcs = xfull
            tsrcs = [i_t0] + tbl

            for l in range(L_RUN):
                last = (l == L_RUN - 1)
                xsrc, tsrc = xsrcs[l], tsrcs[l]
                if not last:
                    stage = sb.tile([P, WPC, 256], dt.bfloat16, tag="stage")
                    nc.vector.memset(stage[:], 0)
                    tstage = sb.tile([P, WPC], dt.float32, tag="tstage")
                else:
                    stagef = sb.tile([P, WPC, D], dt.float32, tag="stage")

                maxw = int(os.environ.get("KGAT_MAXW", str(WPC)))
                w0 = 0
                while w0 < maxw:
                    w1 = min(w0 + SBW, maxw)
                    gc0, gc1 = w0 * C, w1 * C
                    SBC = gc1 - gc0
                    # one [128,1]-offset indirect gather per chunk — the
                    # multi-index form mis-lowers through neuronx_cc here
                    G = sbg.tile([P, SBC, 256], dt.bfloat16, tag="G")
                    TDt = sbg.tile([P, SBC, 1], dt.float32, tag="TD")
                    for c in range(SBC):
                        nc.gpsimd.indirect_dma_start(
                            out=G[:, c, :], out_offset=None, in_=xsrc.ap(),
                            in_offset=bass.IndirectOffsetOnAxis(
                                ap=idx1_sb[:, gc0 + c:gc0 + c + 1], axis=0))
                        nc.gpsimd.indirect_dma_start(
                            out=TDt[:, c, :], out_offset=None, in_=tsrc.ap(),
                            in_offset=bass.IndirectOffsetOnAxis(
                                ap=idx2_sb[:, gc0 + c:gc0 + c + 1], axis=0))
                    Ut = sbg.tile([P, SBC, 1], dt.float32, tag="U")
                    nc.vector.tensor_tensor(out=Ut[:], in0=TDt[:],
                                            in1=G[:, :, D:D + 1],
                                            op=mybir.AluOpType.add)
                    ATT = sbg.tile([P, SBC, 1], dt.float32, tag="ATT")
                    nc.scalar.activation(out=ATT[:], in_=Ut[:],
                                         func=mybir.ActivationFunctionType.Sigmoid)

                    aggp = None
                    for gc in range(gc0, gc1):
                        w, j, c = gc // C, gc % C, gc - gc0
                        Wt = sbw.tile([P, P], dt.bfloat16, tag="W")
                        nc.vector.tensor_scalar(
                            Wt[:], iota_f[:],
                            dla_sb[:, gc:gc + 1], ATT[:, c, 0:1],
                            mybir.AluOpType.is_equal, mybir.AluOpType.mult)
                        if j == 0:
                            aggp = ps.tile([P, P], dt.float32, tag="agg")
                        nc.tensor.matmul(out=aggp[:], lhsT=G[:, c, 0:D], rhs=Wt[:],
                                         start=(j == 0), stop=(j == C - 1))
                        if j != C - 1:
                            continue

                        # ---- window w complete: node update ----
                        aggb = sbw.tile([P, P], dt.bfloat16, tag="aggb")
                        nc.vector.tensor_copy(out=aggb[:], in_=aggp[:])
                        xts = xt_own[:, w * P:(w + 1) * P]
                        up = ps.tile([P, P], dt.float32, tag="up")
                        nc.tensor.matmul(out=up[:],
                                         lhsT=wg_sb[:, (l * 2) * D:(l * 2 + 1) * D],
                                         rhs=xts, start=True, stop=False)
                        nc.tensor.matmul(out=up[:],
                                         lhsT=wg_sb[:, (l * 2 + 1) * D:(l * 2 + 2) * D],
                                         rhs=aggb[:], start=False, stop=True)
                        if not last:
                            nc.scalar.activation(out=xts, in_=up[:],
                                                 func=mybir.ActivationFunctionType.Relu,
                                                 bias=bg_sb[:, l:l + 1])
                            st = ps1.tile([P, 2], dt.float32, tag="st")
                            nc.tensor.matmul(out=st[:], lhsT=xts,
                                             rhs=wast_sb[:, l * 2:l * 2 + 2],
                                             start=True, stop=True)
                            tr = ps1.tile([P, P], dt.bfloat16, tag="tr")
                            nc.tensor.transpose(out=tr[:], in_=xts, identity=ident_b[:])
                            nc.vector.tensor_copy(out=stage[:, w, 0:D], in_=tr[:])
                            nc.scalar.add(out=stage[:, w, D:D + 1], in_=st[:, 0:1],
                                          add=float(ba[l + 1, 0]))
                            nc.vector.tensor_copy(out=tstage[:, w:w + 1], in_=st[:, 1:2])
                        else:
                            xf = sbw.tile([P, P], dt.float32, tag="xf")
                            nc.scalar.activation(out=xf[:], in_=up[:],
                                                 func=mybir.ActivationFunctionType.Relu,
                                                 bias=bg_sb[:, l:l + 1])
                            trf = ps1.tile([P, P], dt.float32, tag="trf")
                            nc.tensor.transpose(out=trf[:], in_=xf[:], identity=ident_f[:])
                            nc.vector.tensor_copy(out=stagef[:, w, :], in_=trf[:])
                    w0 = w1

                if not last:
                    nc.sync.dma_start(
                        out=agin[l + 1].ap().rearrange("(w p) c -> p w c", p=P),
                        in_=stage[:])
                    nc.sync.dma_start(
                        out=tbl[l].ap().rearrange("(w p) o -> p (w o)", p=P),
                        in_=tstage[:])
                    nc.gpsimd.collective_compute(
                        "AllGather", mybir.AluOpType.bypass,
                        replica_groups=[list(range(NCORES))],
                        ins=[agin[l + 1].ap()], outs=[xfull[l + 1].ap()])
                else:
                    nc.sync.dma_start(
                        out=o_out.ap().rearrange("(w p) c -> p w c", p=P),
                        in_=stagef[:])

    nc.compile()
    return nc


def kernel(edge_index, user_emb, item_emb, Wa, ba, Wg, bg):
    global LAST_EXEC_NS
    h = _host_prep(edge_index, user_emb, item_emb, Wa, ba, Wg, bg)
    nc = _build_nc(h["C"], h["NCHUNK"], h["ba"])

    in_maps = []
    for k in range(NCORES):
        in_maps.append({
            "xslab": h["xslab"][k], "xt0": h["xt0"][k], "t0": h["t0a"][k],
            "idx1": h["idx1"][k], "idx2": h["idx2"][k], "dla": h["dla"][k],
            "wg": h["wg_b"], "wast": h["wast"], "bg": h["bg_c"],
        })

    res = run_bass_kernel_spmd(nc, in_maps, core_ids=list(range(NCORES)))
    LAST_EXEC_NS = res.exec_time_ns

    if int(os.environ.get("KGAT_BENCH", "0")):
        LAST_EXEC_NS = _bench(nc, in_maps)

    x = np.zeros((N, D), np.float32)
    for k in range(NCORES):
        x[k * NPC:(k + 1) * NPC] = np.asarray(res.results[k]["out"])[:NPC]
    return x[:U], x[U:]


def _bench(nc, in_maps, iters=6):
    """Time repeated on-device executions via the same PJRT shard_map path
    (device-resident inputs, no donation) and return min wall ns."""
    import time
    import jax
    from jax.sharding import Mesh, PartitionSpec
    from jax.experimental.shard_map import shard_map
    from concourse import bass2jax, mybir as mb

    bass2jax.install_neuronx_cc_hook()
    in_names, out_names, out_avals, zero_outs = [], [], [], []
    for alloc in nc.m.functions[0].allocations:
        if not isinstance(alloc, mb.MemoryLocationSet):
            continue
        name = alloc.memorylocations[0].name
        if alloc.kind == "ExternalInput":
            in_names.append(name)
        elif alloc.kind == "ExternalOutput":
            out_names.append(name)
            shape = tuple(alloc.tensor_shape)
            dtype = mb.dt.np(alloc.dtype)
            out_avals.append(jax.core.ShapedArray(shape, dtype))
            zero_outs.append(np.zeros(shape, dtype))
    n_params = len(in_names)
    all_names = in_names + out_names

    def _body(*args):
        return tuple(bass2jax._bass_exec_p.bind(
            *args, out_avals=tuple(out_avals), in_names=tuple(all_names),
            out_names=tuple(out_names), lowering_input_output_aliases=(),
            sim_require_finite=False, sim_require_nnan=False, nc=nc))

    devices = jax.devices()[:NCORES]
    mesh = Mesh(np.asarray(devices), ("core",))
    specs = (PartitionSpec("core"),) * (n_params + len(out_names))
    fn = jax.jit(shard_map(_body, mesh=mesh, in_specs=specs,
                           out_specs=(PartitionSpec("core"),) * len(out_names),
                           check_rep=False), keep_unused=True)
    concat_in = [np.concatenate([np.asarray(m[n]) for m in in_maps], axis=0)
                 for n in in_names]
    concat_zero = [np.zeros((NCORES * z.shape[0], *z.shape[1:]), z.dtype)
                   for z in zero_outs]
    sharding = jax.sharding.NamedSharding(mesh, PartitionSpec("core"))
    dev_in = [jax.device_put(a, sharding) for a in concat_in + concat_zero]
    jax.block_until_ready(fn(*dev_in))  # warm compile
    best = None
    for _ in range(iters):
        t0 = time.perf_counter()
        jax.block_until_ready(fn(*dev_in))
        dt = time.perf_counter() - t0
        best = dt if best is None else min(best, dt)
    return int(best * 1e9)



# revision 3
# speedup vs baseline: 527.9073x; 527.9073x over previous
"""KGAT recommender (3-layer GNN message passing) on 8 Trainium2 NeuronCores.

Sharding: edges are sharded by destination-node range — core k owns nodes
[k*12500, (k+1)*12500) and aggregates all messages into them, so no
all-reduce is needed; each layer ends with an AllGather of the updated
(bf16) node-embedding table (plus the per-edge attention scalar s=x@Wa_top
appended to each row so edge gathers fetch it for free).

Per 128-edge chunk the attention-weighted segment-sum is computed as a
one-hot matmul: W[e, j] = (j == dst_local[e]) * att[e] built in a single
DVE tensor_scalar op, then PSUM accumulates aggT[d, n] += G[e, d]^T @ W.
"""

import os
import numpy as np
import ml_dtypes

import concourse.bacc as bacc
import concourse.bass as bass
import concourse.mybir as mybir
import concourse.tile as tile
from concourse.bass_utils import run_bass_kernel_spmd
from concourse.masks import make_identity

BF16 = ml_dtypes.bfloat16

NCORES = 8
N = 100000
U = 50000
D = 128
L = 3
P = 128
NPC = N // NCORES          # 12500 nodes per core
WPC = (NPC + P - 1) // P   # 98 windows per core
NSLAB = WPC * P            # 12544 padded rows per core
TAB = NCORES * NSLAB       # 100352 rows in the gather table
SBW = 5                    # windows per superblock

LAST_EXEC_NS = None


def _host_prep(edge_index, user_emb, item_emb, Wa, ba, Wg, bg):
    x0 = np.concatenate([np.asarray(user_emb), np.asarray(item_emb)], 0).astype(np.float32)
    Wa = np.asarray(Wa, np.float32)
    ba = np.asarray(ba, np.float32)
    Wg = np.asarray(Wg, np.float32)
    bg = np.asarray(bg, np.float32)

    src = np.asarray(edge_index[0]).astype(np.int64)
    dst = np.asarray(edge_index[1]).astype(np.int64)
    E = src.shape[0]

    core = dst // NPC
    local = dst % NPC
    w = local // P
    dloc = local % P
    cell = core * WPC + w
    order = np.argsort(cell, kind="stable")
    cell_s = cell[order]
    counts = np.bincount(cell, minlength=NCORES * WPC)
    C = int(np.ceil(counts.max() / P))      # chunks per window (uniform)
    NCHUNK = WPC * C
    starts = np.zeros(NCORES * WPC, np.int64)
    starts[1:] = np.cumsum(counts)[:-1]
    rank = np.arange(E, dtype=np.int64) - starts[cell_s]
    k_arr = cell_s // WPC
    chunk = (cell_s % WPC) * C + rank // P
    p = rank % P

    srcs = src[order]
    idx1 = np.zeros((NCORES, P, NCHUNK), np.int32)
    idx2 = np.zeros((NCORES, P, NCHUNK), np.int32)
    dla = np.full((NCORES, P, NCHUNK), 300.0, np.float32)
    tabrow = (srcs // NPC) * NSLAB + (srcs % NPC)
    idx1[k_arr, p, chunk] = tabrow.astype(np.int32)
    idx2[k_arr, p, chunk] = local[order].astype(np.int32)
    dla[k_arr, p, chunk] = dloc[order].astype(np.float32)

    # layer-0 per-node attention scalars
    s0 = x0 @ Wa[0, :D, 0] + ba[0, 0]
    t0 = x0 @ Wa[0, D:, 0]

    xslab = np.zeros((NCORES, NSLAB, 256), BF16)
    for k in range(NCORES):
        xslab[k, :NPC, :D] = x0[k * NPC:(k + 1) * NPC].astype(BF16)
        xslab[k, :NPC, D] = s0[k * NPC:(k + 1) * NPC].astype(BF16)

    xt0 = np.zeros((NCORES, P, NSLAB), BF16)
    t0a = np.zeros((NCORES, NSLAB, 1), np.float32)
    for k in range(NCORES):
        xp = np.zeros((NSLAB, D), np.float32)
        xp[:NPC] = x0[k * NPC:(k + 1) * NPC]
        xt0[k] = np.ascontiguousarray(xp.T).astype(BF16)
        t0a[k, :NPC, 0] = t0[k * NPC:(k + 1) * NPC]

    wg_b = np.zeros((L, 2, D, D), BF16)
    for l in range(L):
        wg_b[l, 0] = Wg[l, :D].astype(BF16)
        wg_b[l, 1] = Wg[l, D:].astype(BF16)
    wast = np.zeros((L - 1, D, 2), BF16)
    for l in range(1, L):
        wast[l - 1, :, 0] = Wa[l, :D, 0].astype(BF16)
        wast[l - 1, :, 1] = Wa[l, D:, 0].astype(BF16)
    bg_c = bg.reshape(L, D, 1).astype(np.float32)

    return dict(C=C, NCHUNK=NCHUNK, idx1=idx1, idx2=idx2, dla=dla, xslab=xslab,
                xt0=xt0, t0a=t0a, wg_b=wg_b, wast=wast, bg_c=bg_c, ba=ba)


def _build_nc(C, NCHUNK, ba):
    L_RUN = int(os.environ.get("KGAT_LAYERS", str(L)))
    dt = mybir.dt
    nc = bacc.Bacc("TRN2", target_bir_lowering=False, debug=False,
                   enable_asserts=False, num_devices=NCORES)

    i_xslab = nc.dram_tensor("xslab", [NSLAB, 256], dt.bfloat16, kind="ExternalInput")
    i_xt0 = nc.dram_tensor("xt0", [P, NSLAB], dt.bfloat16, kind="ExternalInput")
    i_t0 = nc.dram_tensor("t0", [NSLAB, 1], dt.float32, kind="ExternalInput")
    i_idx1 = nc.dram_tensor("idx1", [P, NCHUNK], dt.int32, kind="ExternalInput")
    i_idx2 = nc.dram_tensor("idx2", [P, NCHUNK], dt.int32, kind="ExternalInput")
    i_dla = nc.dram_tensor("dla", [P, NCHUNK], dt.float32, kind="ExternalInput")
    i_wg = nc.dram_tensor("wg", [L, 2, D, D], dt.bfloat16, kind="ExternalInput")
    i_wast = nc.dram_tensor("wast", [L - 1, D, 2], dt.bfloat16, kind="ExternalInput")
    i_bg = nc.dram_tensor("bg", [L, D, 1], dt.float32, kind="ExternalInput")
    o_out = nc.dram_tensor("out", [NSLAB, D], dt.float32, kind="ExternalOutput")

    agin = [nc.dram_tensor(f"agin{l}", [NSLAB, 256], dt.bfloat16, kind="Internal")
            for l in range(L)]
    xfull = [nc.dram_tensor(f"xfull{l}", [TAB, 256], dt.bfloat16, kind="Internal",
                            addr_space="Shared")
             for l in range(L)]
    tbl = [nc.dram_tensor(f"tbl{l}", [NSLAB, 1], dt.float32, kind="Internal")
           for l in range(L - 1)]

    with tile.TileContext(nc) as tc:
        with (
            tc.tile_pool(name="sb", bufs=1) as sb,
            tc.tile_pool(name="sbg", bufs=2) as sbg,
            tc.tile_pool(name="sbw", bufs=3) as sbw,
            tc.tile_pool(name="ps", bufs=2, space="PSUM") as ps,
            tc.tile_pool(name="ps1", bufs=1, space="PSUM") as ps1,
        ):
            # ---- constants / persistent state ----
            iota_i = sb.tile([P, P], dt.int32)
            nc.gpsimd.iota(iota_i[:], pattern=[[1, P]], base=0, channel_multiplier=0)
            iota_f = sb.tile([P, P], dt.float32)
            nc.vector.tensor_copy(out=iota_f[:], in_=iota_i[:])
            ident_b = sb.tile([P, P], dt.bfloat16)
            make_identity(nc, ident_b[:])
            ident_f = sb.tile([P, P], dt.float32)
            make_identity(nc, ident_f[:])

            idx1_sb = sb.tile([P, NCHUNK], dt.int32)
            nc.sync.dma_start(out=idx1_sb[:], in_=i_idx1.ap())
            idx2_sb = sb.tile([P, NCHUNK], dt.int32)
            nc.sync.dma_start(out=idx2_sb[:], in_=i_idx2.ap())
            dla_sb = sb.tile([P, NCHUNK], dt.float32)
            nc.sync.dma_start(out=dla_sb[:], in_=i_dla.ap())

            wg_sb = sb.tile([P, L * 2 * D], dt.bfloat16)
            for l in range(L):
                for h in range(2):
                    nc.sync.dma_start(out=wg_sb[:, (l * 2 + h) * D:(l * 2 + h + 1) * D],
                                      in_=i_wg.ap()[l, h])
            wast_sb = sb.tile([P, (L - 1) * 2], dt.bfloat16)
            for l in range(L - 1):
                nc.sync.dma_start(out=wast_sb[:, l * 2:l * 2 + 2], in_=i_wast.ap()[l])
            bg_sb = sb.tile([P, L], dt.float32)
            for l in range(L):
                nc.sync.dma_start(out=bg_sb[:, l:l + 1], in_=i_bg.ap()[l])

            xt_own = sb.tile([P, NSLAB], dt.bfloat16)
            nc.sync.dma_start(out=xt_own[:], in_=i_xt0.ap())

            # replicate the layer-0 table: own slab -> AllGather
            nc.sync.dma_start(out=agin[0].ap(), in_=i_xslab.ap())
            nc.gpsimd.collective_compute(
                "AllGather", mybir.AluOpType.bypass,
                replica_groups=[list(range(NCORES))],
                ins=[agin[0].ap()], outs=[xfull[0].ap()])

            xsrcs = xfull
            tsrcs = [i_t0] + tbl

            for l in range(L_RUN):
                last = (l == L_RUN - 1)
                xsrc, tsrc = xsrcs[l], tsrcs[l]
                if not last:
                    stage = sb.tile([P, WPC, 256], dt.bfloat16, tag="stage")
                    nc.vector.memset(stage[:], 0)
                    tstage = sb.tile([P, WPC], dt.float32, tag="tstage")
                else:
                    stagef = sb.tile([P, WPC, D], dt.float32, tag="stage")

                maxw = int(os.environ.get("KGAT_MAXW", str(WPC)))
                w0 = 0
                while w0 < maxw:
                    w1 = min(w0 + SBW, maxw)
                    gc0, gc1 = w0 * C, w1 * C
                    SBC = gc1 - gc0
                    # one [128,1]-offset indirect gather per chunk — the
                    # multi-index form mis-lowers through neuronx_cc here
                    G = sbg.tile([P, SBC, 256], dt.bfloat16, tag="G")
                    TDt = sbg.tile([P, SBC, 1], dt.float32, tag="TD")
                    for c in range(SBC):
                        nc.gpsimd.indirect_dma_start(
                            out=G[:, c, :], out_offset=None, in_=xsrc.ap(),
                            in_offset=bass.IndirectOffsetOnAxis(
                                ap=idx1_sb[:, gc0 + c:gc0 + c + 1], axis=0))
                        nc.gpsimd.indirect_dma_start(
                            out=TDt[:, c, :], out_offset=None, in_=tsrc.ap(),
                            in_offset=bass.IndirectOffsetOnAxis(
                                ap=idx2_sb[:, gc0 + c:gc0 + c + 1], axis=0))
                    Ut = sbg.tile([P, SBC, 1], dt.float32, tag="U")
                    nc.vector.tensor_tensor(out=Ut[:], in0=TDt[:],
                                            in1=G[:, :, D:D + 1],
                                            op=mybir.AluOpType.add)
                    ATT = sbg.tile([P, SBC, 1], dt.float32, tag="ATT")
                    nc.scalar.activation(out=ATT[:], in_=Ut[:],
                                         func=mybir.ActivationFunctionType.Sigmoid)

                    aggp = None
                    for gc in range(gc0, gc1):
                        w, j, c = gc // C, gc % C, gc - gc0
                        Wt = sbw.tile([P, P], dt.bfloat16, tag="W")
                        nc.vector.tensor_scalar(
                            Wt[:], iota_f[:],
                            dla_sb[:, gc:gc + 1], ATT[:, c, 0:1],
                            mybir.AluOpType.is_equal, mybir.AluOpType.mult)
                        if j == 0:
                            aggp = ps.tile([P, P], dt.float32, tag="agg")
                        nc.tensor.matmul(out=aggp[:], lhsT=G[:, c, 0:D], rhs=Wt[:],
                                         start=(j == 0), stop=(j == C - 1))
                        if j != C - 1:
                            continue

                        # ---- window w complete: node update ----
                        aggb = sbw.tile([P, P], dt.bfloat16, tag="aggb")
                        nc.vector.tensor_copy(out=aggb[:], in_=aggp[:])
                        xts = xt_own[:, w * P:(w + 1) * P]
                        up = ps.tile([P, P], dt.float32, tag="up")
                        nc.tensor.matmul(out=up[:],
                                         lhsT=wg_sb[:, (l * 2) * D:(l * 2 + 1) * D],
                                         rhs=xts, start=True, stop=False)
                        nc.tensor.matmul(out=up[:],
                                         lhsT=wg_sb[:, (l * 2 + 1) * D:(l * 2 + 2) * D],
                                         rhs=aggb[:], start=False, stop=True)
                        if not last:
                            nc.scalar.activation(out=xts, in_=up[:],
                                                 func=mybir.ActivationFunctionType.Relu,
                                                 bias=bg_sb[:, l:l + 1])
                            st = ps1.tile([P, 2], dt.float32, tag="st")
                            nc.tensor.matmul(out=st[:], lhsT=xts,
                                             rhs=wast_sb[:, l * 2:l * 2 + 2],
                                             start=True, stop=True)
                            tr = ps1.tile([P, P], dt.bfloat16, tag="tr")
                            nc.tensor.transpose(out=tr[:], in_=xts, identity=ident_b[:])
                            nc.vector.tensor_copy(out=stage[:, w, 0:D], in_=tr[:])
                            nc.scalar.add(out=stage[:, w, D:D + 1], in_=st[:, 0:1],
                                          add=float(ba[l + 1, 0]))
                            nc.vector.tensor_copy(out=tstage[:, w:w + 1], in_=st[:, 1:2])
                        else:
                            xf = sbw.tile([P, P], dt.float32, tag="xf")
                            nc.scalar.activation(out=xf[:], in_=up[:],
                                                 func=mybir.ActivationFunctionType.Relu,
                                                 bias=bg_sb[:, l:l + 1])
                            trf = ps1.tile([P, P], dt.float32, tag="trf")
                            nc.tensor.transpose(out=trf[:], in_=xf[:], identity=ident_f[:])
                            nc.vector.tensor_copy(out=stagef[:, w, :], in_=trf[:])
                    w0 = w1

                if not last:
                    nc.sync.dma_start(
                        out=agin[l + 1].ap().rearrange("(w p) c -> p w c", p=P),
                        in_=stage[:])
                    nc.sync.dma_start(
                        out=tbl[l].ap().rearrange("(w p) o -> p (w o)", p=P),
                        in_=tstage[:])
                    nc.gpsimd.collective_compute(
                        "AllGather", mybir.AluOpType.bypass,
                        replica_groups=[list(range(NCORES))],
                        ins=[agin[l + 1].ap()], outs=[xfull[l + 1].ap()])
                else:
                    nc.sync.dma_start(
                        out=o_out.ap().rearrange("(w p) c -> p w c", p=P),
                        in_=stagef[:])

    nc.compile()
    return nc


def kernel(edge_index, user_emb, item_emb, Wa, ba, Wg, bg):
    global LAST_EXEC_NS
    h = _host_prep(edge_index, user_emb, item_emb, Wa, ba, Wg, bg)
    nc = _build_nc(h["C"], h["NCHUNK"], h["ba"])

    in_maps = []
    for k in range(NCORES):
        in_maps.append({
            "xslab": h["xslab"][k], "xt0": h["xt0"][k], "t0": h["t0a"][k],
            "idx1": h["idx1"][k], "idx2": h["idx2"][k], "dla": h["dla"][k],
            "wg": h["wg_b"], "wast": h["wast"], "bg": h["bg_c"],
        })

    res = run_bass_kernel_spmd(nc, in_maps, core_ids=list(range(NCORES)))
    LAST_EXEC_NS = res.exec_time_ns

    if int(os.environ.get("KGAT_BENCH", "0")):
        LAST_EXEC_NS = _bench(nc, in_maps)

    x = np.zeros((N, D), np.float32)
    for k in range(NCORES):
        x[k * NPC:(k + 1) * NPC] = np.asarray(res.results[k]["out"])[:NPC]
    return x[:U], x[U:]


def _bench(nc, in_maps, iters=6):
    """Time repeated on-device executions via the same PJRT shard_map path
    (device-resident inputs, no donation) and return min wall ns."""
    import time
    import jax
    from jax.sharding import Mesh, PartitionSpec
    from jax.experimental.shard_map import shard_map
    from concourse import bass2jax, mybir as mb

    bass2jax.install_neuronx_cc_hook()
    part_name = nc.partition_id_tensor.name if nc.partition_id_tensor else None
    in_names, out_names, out_avals, zero_outs = [], [], [], []
    for alloc in nc.m.functions[0].allocations:
        if not isinstance(alloc, mb.MemoryLocationSet):
            continue
        name = alloc.memorylocations[0].name
        if alloc.kind == "ExternalInput":
            if name != part_name:
                in_names.append(name)
        elif alloc.kind == "ExternalOutput":
            out_names.append(name)
            shape = tuple(alloc.tensor_shape)
            dtype = mb.dt.np(alloc.dtype)
            out_avals.append(jax.core.ShapedArray(shape, dtype))
            zero_outs.append(np.zeros(shape, dtype))
    n_params = len(in_names)
    all_names = in_names + out_names
    if part_name is not None:
        all_names = all_names + [part_name]

    def _body(*args):
        operands = list(args)
        if part_name is not None:
            operands.append(bass2jax.partition_id_tensor())
        return tuple(bass2jax._bass_exec_p.bind(
            *operands, out_avals=tuple(out_avals), in_names=tuple(all_names),
            out_names=tuple(out_names), lowering_input_output_aliases=(),
            sim_require_finite=False, sim_require_nnan=False, nc=nc))

    devices = jax.devices()[:NCORES]
    mesh = Mesh(np.asarray(devices), ("core",))
    specs = (PartitionSpec("core"),) * (n_params + len(out_names))
    fn = jax.jit(shard_map(_body, mesh=mesh, in_specs=specs,
                           out_specs=(PartitionSpec("core"),) * len(out_names),
                           check_rep=False), keep_unused=True)
    concat_in = [np.concatenate([np.asarray(m[n]) for m in in_maps], axis=0)
                 for n in in_names]
    concat_zero = [np.zeros((NCORES * z.shape[0], *z.shape[1:]), z.dtype)
                   for z in zero_outs]
    sharding = jax.sharding.NamedSharding(mesh, PartitionSpec("core"))
    dev_in = [jax.device_put(a, sharding) for a in concat_in + concat_zero]
    jax.block_until_ready(fn(*dev_in))  # warm compile
    best = None
    for _ in range(iters):
        t0 = time.perf_counter()
        jax.block_until_ready(fn(*dev_in))
        dt = time.perf_counter() - t0
        best = dt if best is None else min(best, dt)
    return int(best * 1e9)



# revision 12
# speedup vs baseline: 544.7472x; 1.0319x over previous
"""KGAT recommender (3-layer GNN message passing) on 8 Trainium2 NeuronCores.

Sharding: edges are sharded by destination-node range — core k owns nodes
[k*12500, (k+1)*12500) and aggregates all messages into them; each layer ends
with an AllGather of the updated (bf16) node-embedding table with the per-node
attention scalar s appended to each 512B row.

Gathers use InstDMAGatherAnt (dma_gather, ~1024 idxs per call = the SWDGE
descriptor-ring cap): edges are grouped per (superblock of SBW dst-windows,
32768-row source range so idx fits int16) and packed densely into 128-edge
chunks. The per-(range,window) edge count is uniformized to the max across
cores so one SPMD program fits all 8 cores; per-core pads gather row 0 and
carry dla=300 so the one-hot mask kills them.

Chunks may straddle window boundaries: the attention-weighted one-hot
scatter matmul runs per segment (host-prepped dla mask columns), PSUM
accumulates per window across the superblock's four range groups.

att = sigmoid(s[src] + t[dst]): s rides the gathered row (col 128); t is
local per dst window — its column is free-axis-broadcast, PE-transposed to
a [edge, node] tile, and the per-edge s is added via the ACT bias port
inside a single Sigmoid op per chunk.
"""

import os
import math
import numpy as np
import ml_dtypes

import concourse.bacc as bacc
import concourse.bass as bass
import concourse.mybir as mybir
import concourse.tile as tile
from concourse.bass_utils import run_bass_kernel_spmd
from concourse.masks import make_identity

BF16 = ml_dtypes.bfloat16

NCORES = 8
N = 100000
U = 50000
D = 128
L = 3
P = 128
NPC = N // NCORES          # 12500 nodes per core
WPC = (NPC + P - 1) // P   # 98 windows per core
NSLAB = WPC * P            # 12544 padded rows per core
TAB = NCORES * NSLAB       # 100352 rows in the gather table
SBW = 4                    # windows per superblock
NSB = (WPC + SBW - 1) // SBW
NRANGE = 4
RSIZE = 32768              # int16-addressable rows per dma_gather call
CALLCH = 8                 # max chunks (=1024 idxs) per dma_gather call

LAST_EXEC_NS = None


def _global_structure(n_uni):
    """n_uni: [NRANGE, WPC] uniform edge counts. Returns the shared emission
    structure: per sb, per g: chunk count, calls, chunk segments, and the
    packed-space offsets; plus total idx cols / segment count."""
    struct = []
    icol = 0
    nseg = 0
    space = 0
    for sb in range(NSB):
        ws = list(range(sb * SBW, min((sb + 1) * SBW, WPC)))
        gmetas = []
        for g in range(NRANGE):
            total = int(sum(n_uni[g, w] for w in ws))
            nch = (total + P - 1) // P
            # position -> window map over the packed space
            wmap = np.full(nch * P, -1, np.int64)
            woff = {}
            o = 0
            for w in ws:
                nw = int(n_uni[g, w])
                woff[w] = o
                wmap[o:o + nw] = w
                o += nw
            chunks = []
            for c in range(nch):
                segs = []
                cs = wmap[c * P:(c + 1) * P]
                i = 0
                while i < P:
                    if cs[i] < 0:
                        i += 1
                        continue
                    j = i
                    while j < P and cs[j] == cs[i]:
                        j += 1
                    segs.append(dict(w=int(cs[i]), lo=i, hi=j, col=nseg))
                    nseg += 1
                    i = j
                chunks.append(dict(segs=segs))
            calls = []
            for clo in range(0, nch, CALLCH):
                chi = min(clo + CALLCH, nch)
                ni = (chi - clo) * P
                calls.append(dict(clo=clo, chi=chi, icol=icol, ncol=ni // 16))
                icol += ni // 16
            gmetas.append(dict(nch=nch, calls=calls, chunks=chunks,
                               woff=woff, offset=space))
            space += nch * P
        struct.append(dict(ws=ws, gs=gmetas))
    return struct, icol, max(nseg, 1), space


def _host_prep(edge_index, user_emb, item_emb, Wa, ba, Wg, bg):
    x0 = np.concatenate([np.asarray(user_emb), np.asarray(item_emb)], 0).astype(np.float32)
    Wa = np.asarray(Wa, np.float32)
    ba = np.asarray(ba, np.float32)
    Wg = np.asarray(Wg, np.float32)
    bg = np.asarray(bg, np.float32)

    src = np.asarray(edge_index[0]).astype(np.int64)
    dst = np.asarray(edge_index[1]).astype(np.int64)

    core = dst // NPC
    local = dst % NPC
    w_all = local // P
    dla_all = (local % P).astype(np.float32)
    tabrow = (src // NPC) * NSLAB + (src % NPC)
    g_all = tabrow // RSIZE
    ridx_all = (tabrow % RSIZE).astype(np.int16)

    counts = np.zeros((NCORES, NRANGE, WPC), np.int64)
    np.add.at(counts, (core, g_all, w_all), 1)
    # round up to 32 so window boundaries inside chunks are 32-aligned
    # (partition-sliced engine ops require 32-aligned partition bases)
    n_uni = (counts.max(axis=0) + 31) // 32 * 32

    struct, IDXC_RAW, NSEG, SPACE = _global_structure(n_uni)

    # ---- per-core packed idx / dla fills ----
    idx_cores = []
    dla_cores = []
    for k in range(NCORES):
        sel = np.nonzero(core == k)[0]
        gv, wv, dlav, riv = g_all[sel], w_all[sel], dla_all[sel], ridx_all[sel]
        sbv = wv // SBW
        order = np.lexsort((wv, gv, sbv))
        gv, wv, dlav, riv = gv[order], wv[order], dlav[order], riv[order]
        fidx = np.zeros(SPACE, np.int16)
        fdla = np.full(SPACE, 300.0, np.float32)
        pos = 0
        for sb in range(NSB):
            for g in range(NRANGE):
                gm = struct[sb]["gs"][g]
                for w in struct[sb]["ws"]:
                    nk = int(counts[k, g, w])
                    if nk == 0:
                        continue
                    start = gm["offset"] + gm["woff"][w]
                    fidx[start:start + nk] = riv[pos:pos + nk]
                    fdla[start:start + nk] = dlav[pos:pos + nk]
                    pos += nk
        assert pos == len(sel)
        # idx tensor: per call, wrapped [16, ncol] then tiled to 128
        blocks = []
        for sb in range(NSB):
            for g in range(NRANGE):
                gm = struct[sb]["gs"][g]
                for call in gm["calls"]:
                    ni = (call["chi"] - call["clo"]) * P
                    blk = fidx[gm["offset"] + call["clo"] * P:
                               gm["offset"] + call["chi"] * P]
                    blocks.append(blk.reshape(ni // 16, 16).T)
        idx_arr = np.tile(np.concatenate(blocks, 1), (8, 1)).copy()
        # dla tensor: one column per segment
        dla_arr = np.full((P, NSEG), 300.0, np.float32)
        for sb in range(NSB):
            for g in range(NRANGE):
                gm = struct[sb]["gs"][g]
                for c, ch in enumerate(gm["chunks"]):
                    base = gm["offset"] + c * P
                    for s in ch["segs"]:
                        dla_arr[s["lo"]:s["hi"], s["col"]] = \
                            fdla[base + s["lo"]:base + s["hi"]]
        idx_cores.append(idx_arr)
        dla_cores.append(dla_arr)

    # ---- layer-0 tables ----
    s0 = x0 @ Wa[0, :D, 0] + ba[0, 0]
    t0 = x0 @ Wa[0, D:, 0]

    xslab = np.zeros((NCORES, NSLAB, 256), BF16)
    xt0 = np.zeros((NCORES, P, NSLAB), BF16)
    t0w = np.zeros((NCORES, P, WPC), np.float32)
    for k in range(NCORES):
        xslab[k, :NPC, :D] = x0[k * NPC:(k + 1) * NPC].astype(BF16)
        xslab[k, :NPC, D] = s0[k * NPC:(k + 1) * NPC].astype(BF16)
        xp = np.zeros((NSLAB, D), np.float32)
        xp[:NPC] = x0[k * NPC:(k + 1) * NPC]
        xt0[k] = np.ascontiguousarray(xp.T).astype(BF16)
        tp = np.zeros(NSLAB, np.float32)
        tp[:NPC] = t0[k * NPC:(k + 1) * NPC]
        t0w[k] = tp.reshape(WPC, P).T

    wg_b = np.zeros((L, 2, D, D), BF16)
    for l in range(L):
        wg_b[l, 0] = Wg[l, :D].astype(BF16)
        wg_b[l, 1] = Wg[l, D:].astype(BF16)
    wast = np.zeros((L - 1, D, 2), BF16)
    for l in range(1, L):
        wast[l - 1, :, 0] = Wa[l, :D, 0].astype(BF16)
        wast[l - 1, :, 1] = Wa[l, D:, 0].astype(BF16)
    bg_c = bg.reshape(L, D, 1).astype(np.float32)

    GCH = max(gm["nch"] for s in struct for gm in s["gs"])
    return dict(struct=struct, IDXC=idx_cores[0].shape[1], NSEG=NSEG, GCH=GCH,
                idx_cores=idx_cores, dla_cores=dla_cores,
                xslab=xslab, xt0=xt0, t0w=t0w, wg_b=wg_b, wast=wast,
                bg_c=bg_c, ba=ba)


def _build_nc(h):
    struct = h["struct"]
    IDXC, NSEG, GCH = h["IDXC"], h["NSEG"], h["GCH"]
    ba = h["ba"]
    L_RUN = int(os.environ.get("KGAT_LAYERS", str(L)))
    REPS = int(os.environ.get("KGAT_REPS", "1"))
    dt = mybir.dt
    nc = bacc.Bacc("TRN2", target_bir_lowering=False, debug=False,
                   enable_asserts=False, num_devices=NCORES)

    i_xslab = nc.dram_tensor("xslab", [NSLAB, 256], dt.bfloat16, kind="ExternalInput")
    i_xt0 = nc.dram_tensor("xt0", [P, NSLAB], dt.bfloat16, kind="ExternalInput")
    i_t0w = nc.dram_tensor("t0w", [P, WPC], dt.float32, kind="ExternalInput")
    i_idx = nc.dram_tensor("idx", [P, IDXC], dt.int16, kind="ExternalInput")
    i_dla = nc.dram_tensor("dla", [P, NSEG], dt.float32, kind="ExternalInput")
    i_wg = nc.dram_tensor("wg", [L, 2, D, D], dt.bfloat16, kind="ExternalInput")
    i_wast = nc.dram_tensor("wast", [L - 1, D, 2], dt.bfloat16, kind="ExternalInput")
    i_bg = nc.dram_tensor("bg", [L, D, 1], dt.float32, kind="ExternalInput")
    o_out = nc.dram_tensor("out", [NSLAB, D], dt.float32, kind="ExternalOutput")

    agin = [nc.dram_tensor(f"agin{l}", [NSLAB, 256], dt.bfloat16, kind="Internal")
            for l in range(L)]
    xfull = [nc.dram_tensor(f"xfull{l}", [TAB, 256], dt.bfloat16, kind="Internal",
                            addr_space="Shared")
             for l in range(L)]

    with tile.TileContext(nc) as tc:
        with (
            tc.tile_pool(name="sb", bufs=1) as sbp,
            tc.tile_pool(name="gp", bufs=8) as gp,
            tc.tile_pool(name="tbp", bufs=10) as tbp,
            tc.tile_pool(name="wk", bufs=4) as wk,
            tc.tile_pool(name="pa", bufs=4, space="PSUM") as pa,
            tc.tile_pool(name="pu", bufs=2, space="PSUM") as pu,
            tc.tile_pool(name="pt", bufs=2, space="PSUM") as pt,
        ):
            # ---- constants / persistent state ----
            iota_i = sbp.tile([P, P], dt.int32)
            nc.gpsimd.iota(iota_i[:], pattern=[[1, P]], base=0, channel_multiplier=0)
            iota_f = sbp.tile([P, P], dt.float32)
            nc.vector.tensor_copy(out=iota_f[:], in_=iota_i[:])
            ident_b = sbp.tile([P, P], dt.bfloat16)
            make_identity(nc, ident_b[:])
            ident_f = sbp.tile([P, P], dt.float32)
            make_identity(nc, ident_f[:])

            idx_sb = sbp.tile([P, IDXC], dt.int16)
            nc.sync.dma_start(out=idx_sb[:], in_=i_idx.ap())
            dla_sb = sbp.tile([P, NSEG], dt.float32)
            nc.sync.dma_start(out=dla_sb[:], in_=i_dla.ap())

            wg_sb = sbp.tile([P, L * 2 * D], dt.bfloat16)
            for l in range(L):
                for hh in range(2):
                    nc.sync.dma_start(out=wg_sb[:, (l * 2 + hh) * D:(l * 2 + hh + 1) * D],
                                      in_=i_wg.ap()[l, hh])
            wast_sb = sbp.tile([P, (L - 1) * 2], dt.bfloat16)
            for l in range(L - 1):
                nc.sync.dma_start(out=wast_sb[:, l * 2:l * 2 + 2], in_=i_wast.ap()[l])
            bg_sb = sbp.tile([P, L], dt.float32)
            for l in range(L):
                nc.sync.dma_start(out=bg_sb[:, l:l + 1], in_=i_bg.ap()[l])

            xt_own = sbp.tile([P, NSLAB], dt.bfloat16)
            nc.sync.dma_start(out=xt_own[:], in_=i_xt0.ap())
            t0_sb = sbp.tile([P, WPC], dt.float32)
            nc.sync.dma_start(out=t0_sb[:], in_=i_t0w.ap())

            # replicate the layer-0 table: own slab -> AllGather
            nc.sync.dma_start(out=agin[0].ap(), in_=i_xslab.ap())
            nc.gpsimd.collective_compute(
                "AllGather", mybir.AluOpType.bypass,
                replica_groups=[list(range(NCORES))],
                ins=[agin[0].ap()], outs=[xfull[0].ap()])

            tst_tiles = [t0_sb] + [sbp.tile([P, WPC], dt.float32, tag=f"tst{l}",
                                            name=f"tst{l}")
                                   for l in range(1, L)]

            for rep in range(REPS):
                if rep > 0:  # timing-only amplification: re-run identically
                    nc.sync.dma_start(out=xt_own[:], in_=i_xt0.ap())
                for l in range(L_RUN):
                    last = (l == L_RUN - 1)
                    xsrc = xfull[l]
                    tst_cur = tst_tiles[l]
                    if not last:
                        tst_next = tst_tiles[l + 1]
                        stage = sbp.tile([P, WPC, 256], dt.bfloat16, tag="stage")
                    else:
                        stagef = sbp.tile([P, WPC, D], dt.float32, tag="stagef")

                    for sb in range(NSB):
                        ws = struct[sb]["ws"]
                        nw = len(ws)
                        # ---- gathers: one G tile + <=CALLCH-chunk calls per range ----
                        gtiles = []
                        for g in range(NRANGE):
                            gm = struct[sb]["gs"][g]
                            G = gp.tile([P, GCH, 256], dt.bfloat16, tag="G")
                            gtiles.append(G)
                            rlo = g * RSIZE
                            rhi = min(rlo + RSIZE, TAB)
                            for call in gm["calls"]:
                                ni = (call["chi"] - call["clo"]) * P
                                nc.gpsimd.dma_gather(
                                    out_ap=G[:, call["clo"]:call["chi"], :],
                                    in_ap=xsrc.ap()[rlo:rhi, :],
                                    idxs_ap=idx_sb[:, call["icol"]:call["icol"] + call["ncol"]],
                                    num_idxs=ni, num_idxs_reg=ni, elem_size=256)
                        # ---- s columns (ACT reads the gather tiles) ----
                        scol = wk.tile([P, NRANGE * CALLCH * 2], dt.float32, tag="scol")
                        goffs = []
                        off = 0
                        for g in range(NRANGE):
                            nch = struct[sb]["gs"][g]["nch"]
                            goffs.append(off)
                            if nch:
                                nc.scalar.add(out=scol[:, off:off + nch],
                                              in_=gtiles[g][:, 0:nch, 128], add=0.0)
                            off += nch
                        # ---- per-window t tiles: broadcast + transpose ----
                        tb_tiles = {}
                        for wi, w in enumerate(ws):
                            tbT = wk.tile([P, P], dt.bfloat16, tag="tbT")
                            nc.vector.tensor_copy(
                                out=tbT[:], in_=tst_cur[:, w:w + 1].to_broadcast([P, P]))
                            tps = pt.tile([P, P], dt.bfloat16, tag="t2", name="tps")
                            nc.tensor.transpose(out=tps[:], in_=tbT[:], identity=ident_b[:])
                            tb = tbp.tile([P, P], dt.bfloat16, tag="tb")
                            nc.vector.tensor_copy(out=tb[:], in_=tps[:])
                            tb_tiles[w] = tb
                        # ---- chunk processing ----
                        aggs = {w: None for w in ws}
                        # first/last segment bookkeeping per window
                        seg_count = {w: 0 for w in ws}
                        for g in range(NRANGE):
                            for ch in struct[sb]["gs"][g]["chunks"]:
                                for s in ch["segs"]:
                                    seg_count[s["w"]] += 1
                        seg_seen = {w: 0 for w in ws}
                        for g in range(NRANGE):
                            gm = struct[sb]["gs"][g]
                            G = gtiles[g]
                            for c, ch in enumerate(gm["chunks"]):
                                segs = ch["segs"]
                                if not segs:
                                    continue
                                # t input tile for the sigmoid
                                if len(segs) == 1:
                                    tb_in = tb_tiles[segs[0]["w"]]
                                else:
                                    # 32-aligned quarters, each window-pure
                                    tb_in = tbp.tile([P, P], dt.bfloat16, tag="tbc")
                                    for q in range(4):
                                        qlo = q * 32
                                        wq = segs[-1]["w"]
                                        for s in segs:
                                            if s["lo"] <= qlo < s["hi"]:
                                                wq = s["w"]
                                                break
                                        nc.vector.tensor_copy(
                                            out=tb_in[qlo:qlo + 32, :],
                                            in_=tb_tiles[wq][qlo:qlo + 32, :])
                                sig = wk.tile([P, P], dt.bfloat16, tag="sig")
                                nc.scalar.activation(
                                    out=sig[:], in_=tb_in[:],
                                    func=mybir.ActivationFunctionType.Sigmoid,
                                    bias=scol[:, goffs[g] + c:goffs[g] + c + 1])
                                for s in segs:
                                    w = s["w"]
                                    O = wk.tile([P, P], dt.bfloat16, tag="O")
                                    nc.vector.tensor_scalar(
                                        O[:], iota_f[:], dla_sb[:, s["col"]:s["col"] + 1],
                                        None, mybir.AluOpType.is_equal)
                                    Wt = wk.tile([P, P], dt.bfloat16, tag="Wt")
                                    nc.vector.tensor_tensor(
                                        out=Wt[:], in0=O[:], in1=sig[:],
                                        op=mybir.AluOpType.mult)
                                    if seg_seen[w] == 0:
                                        aggs[w] = pa.tile([P, P], dt.float32,
                                                          tag="agg", name="aggw")
                                    nc.tensor.matmul(
                                        out=aggs[w][:], lhsT=G[:, c, 0:D], rhs=Wt[:],
                                        start=(seg_seen[w] == 0),
                                        stop=(seg_seen[w] == seg_count[w] - 1))
                                    seg_seen[w] += 1
                        # ---- window updates ----
                        for w in ws:
                            aggb = wk.tile([P, P], dt.bfloat16, tag="aggb")
                            if aggs[w] is None:
                                nc.vector.memset(aggb[:], 0)
                            else:
                                nc.vector.tensor_copy(out=aggb[:], in_=aggs[w][:])
                            xts = xt_own[:, w * P:(w + 1) * P]
                            up = pu.tile([P, P], dt.float32, tag="u2", name="up")
                            nc.tensor.matmul(out=up[:],
                                             lhsT=wg_sb[:, (l * 2) * D:(l * 2 + 1) * D],
                                             rhs=xts, start=True, stop=False)
                            nc.tensor.matmul(out=up[:],
                                             lhsT=wg_sb[:, (l * 2 + 1) * D:(l * 2 + 2) * D],
                                             rhs=aggb[:], start=False, stop=True)
                            if not last:
                                nc.scalar.activation(out=xts, in_=up[:],
                                                     func=mybir.ActivationFunctionType.Relu,
                                                     bias=bg_sb[:, l:l + 1])
                                st = pu.tile([P, P], dt.float32, tag="u2", name="st")
                                nc.tensor.matmul(out=st[:, 0:2], lhsT=xts,
                                                 rhs=wast_sb[:, l * 2:l * 2 + 2],
                                                 start=True, stop=True)
                                tr = pt.tile([P, P], dt.bfloat16, tag="t2", name="tr")
                                nc.tensor.transpose(out=tr[:], in_=xts, identity=ident_b[:])
                                nc.vector.tensor_copy(out=stage[:, w, 0:D], in_=tr[:])
                                nc.scalar.add(out=stage[:, w, D:D + 1], in_=st[:, 0:1],
                                              add=float(ba[l + 1, 0]))
                                nc.vector.tensor_copy(out=tst_next[:, w:w + 1],
                                                      in_=st[:, 1:2])
                            else:
                                xf = wk.tile([P, P], dt.float32, tag="xf")
                                nc.scalar.activation(out=xf[:], in_=up[:],
                                                     func=mybir.ActivationFunctionType.Relu,
                                                     bias=bg_sb[:, l:l + 1])
                                trf = pu.tile([P, P], dt.float32, tag="u2", name="trf")
                                nc.tensor.transpose(out=trf[:], in_=xf[:],
                                                    identity=ident_f[:])
                                nc.vector.tensor_copy(out=stagef[:, w, :], in_=trf[:])

                    if not last:
                        nc.sync.dma_start(
                            out=agin[l + 1].ap().rearrange("(w p) c -> p w c", p=P),
                            in_=stage[:])
                        nc.gpsimd.collective_compute(
                            "AllGather", mybir.AluOpType.bypass,
                            replica_groups=[list(range(NCORES))],
                            ins=[agin[l + 1].ap()], outs=[xfull[l + 1].ap()])
                    else:
                        nc.sync.dma_start(
                            out=o_out.ap().rearrange("(w p) c -> p w c", p=P),
                            in_=stagef[:])

    nc.compile()
    return nc


def kernel(edge_index, user_emb, item_emb, Wa, ba, Wg, bg):
    global LAST_EXEC_NS
    h = _host_prep(edge_index, user_emb, item_emb, Wa, ba, Wg, bg)
    nc = _build_nc(h)

    in_maps = []
    for k in range(NCORES):
        in_maps.append({
            "xslab": h["xslab"][k], "xt0": h["xt0"][k], "t0w": h["t0w"][k],
            "idx": h["idx_cores"][k], "dla": h["dla_cores"][k],
            "wg": h["wg_b"], "wast": h["wast"], "bg": h["bg_c"],
        })

    res = run_bass_kernel_spmd(nc, in_maps, core_ids=list(range(NCORES)))
    LAST_EXEC_NS = res.exec_time_ns

    if int(os.environ.get("KGAT_BENCH", "0")):
        LAST_EXEC_NS = _bench(nc, in_maps)

    x = np.zeros((N, D), np.float32)
    for k in range(NCORES):
        x[k * NPC:(k + 1) * NPC] = np.asarray(res.results[k]["out"])[:NPC]
    return x[:U], x[U:]


def _bench(nc, in_maps, iters=6):
    """Time repeated on-device executions via the same PJRT shard_map path
    (device-resident inputs, no donation) and return min wall ns."""
    import time
    import jax
    from jax.sharding import Mesh, PartitionSpec
    from jax.experimental.shard_map import shard_map
    from concourse import bass2jax, mybir as mb

    bass2jax.install_neuronx_cc_hook()
    part_name = nc.partition_id_tensor.name if nc.partition_id_tensor else None
    in_names, out_names, out_avals, zero_outs = [], [], [], []
    for alloc in nc.m.functions[0].allocations:
        if not isinstance(alloc, mb.MemoryLocationSet):
            continue
        name = alloc.memorylocations[0].name
        if alloc.kind == "ExternalInput":
            if name != part_name:
                in_names.append(name)
        elif alloc.kind == "ExternalOutput":
            out_names.append(name)
            shape = tuple(alloc.tensor_shape)
            dtype = mb.dt.np(alloc.dtype)
            out_avals.append(jax.core.ShapedArray(shape, dtype))
            zero_outs.append(np.zeros(shape, dtype))
    n_params = len(in_names)
    all_names = in_names + out_names
    if part_name is not None:
        all_names = all_names + [part_name]

    def _body(*args):
        operands = list(args)
        if part_name is not None:
            operands.append(bass2jax.partition_id_tensor())
        return tuple(bass2jax._bass_exec_p.bind(
            *operands, out_avals=tuple(out_avals), in_names=tuple(all_names),
            out_names=tuple(out_names), lowering_input_output_aliases=(),
            sim_require_finite=False, sim_require_nnan=False, nc=nc))

    devices = jax.devices()[:NCORES]
    mesh = Mesh(np.asarray(devices), ("core",))
    specs = (PartitionSpec("core"),) * (n_params + len(out_names))
    fn = jax.jit(shard_map(_body, mesh=mesh, in_specs=specs,
                           out_specs=(PartitionSpec("core"),) * len(out_names),
                           check_rep=False), keep_unused=True)
    concat_in = [np.concatenate([np.asarray(m[n]) for m in in_maps], axis=0)
                 for n in in_names]
    concat_zero = [np.zeros((NCORES * z.shape[0], *z.shape[1:]), z.dtype)
                   for z in zero_outs]
    sharding = jax.sharding.NamedSharding(mesh, PartitionSpec("core"))
    dev_in = [jax.device_put(a, sharding) for a in concat_in + concat_zero]
    jax.block_until_ready(fn(*dev_in))  # warm compile
    best = None
    for _ in range(iters):
        t0 = time.perf_counter()
        jax.block_until_ready(fn(*dev_in))
        dt = time.perf_counter() - t0
        best = dt if best is None else min(best, dt)
    return int(best * 1e9)
